# revision 14
# baseline (speedup 1.0000x reference)
"""Trainium2 Bass kernel for nn_ARSLMModel (2-layer gated recurrent LM).

Strategy (8 NeuronCores):
  - The head output [B,S,V] (1 GB fp32) dominates -> shard the vocab dim
    across cores (V/8 = 4000 per core). Host concatenates.
  - The 256-step recurrence is tiny compute but strictly sequential; it is
    replicated on every core (sharding batch would not reduce per-step
    instruction count) and overlapped with the head matmul + output DMA.
  - Matmuls run in bf16 (validated: end-to-end l2 rel err ~5e-3 vs fp32
    reference, gate 2e-2); all elementwise math in fp32.
  - LayerNorm rsqrt via bit-trick seed + Newton iterations on DVE (ACT table
    stays resident in the sigmoid set: relu/sigmoid/square/copy only).

Layouts:
  - Hidden state kept B-major [32, 64] for all elementwise/LN work; matmuls
    take the transposed state [64, 32] (DVE 32x32 stream transposes) as the
    stationary operand so outputs land B-major.
  - Layer-2 transposed states are written straight into a [64, 128] staging
    tile covering 4 timesteps; that tile IS the head matmul's stationary
    operand (rows r = (t%4)*32 + b match the SB-major output order).
"""

import numpy as np

import concourse.bass as bass
import concourse.mybir as mybir
from concourse import bacc, tile
from concourse.masks import make_identity
from concourse.bass_utils import run_bass_kernel_spmd

V, E, H, L = 32000, 64, 64, 2
B, S = 32, 256
NCORES = 8
VC = V // NCORES

F32 = mybir.dt.float32
BF16 = mybir.dt.bfloat16
I32 = mybir.dt.int32
AL = mybir.AluOpType
AF = mybir.ActivationFunctionType

NEWTON_ITERS = 1
MAGIC = 0x5F3759DF

_BUILD_CACHE = {}


def _build(n_steps, with_bias, with_ln_affine):
    """Build the SPMD single-core graph (all cores run the same program)."""
    nc = bacc.Bacc()

    xsb_d = nc.declare_dram_parameter("xsb", [n_steps * B, E], F32, isOutput=False)
    w1c_d = nc.declare_dram_parameter("w1c", [L, 3, H, H + 1], F32, isOutput=False)
    w2_d = nc.declare_dram_parameter("w2", [L, H, H], F32, isOutput=False)
    hw_d = nc.declare_dram_parameter("headw", [H, VC], F32, isOutput=False)
    out_d = nc.declare_dram_parameter("out", [B, n_steps, VC], BF16, isOutput=True)
    if with_bias:
        b1c_d = nc.declare_dram_parameter("b1c", [L, H + 1], F32, isOutput=False)
        b2_d = nc.declare_dram_parameter("b2v", [L, H], F32, isOutput=False)
        hb_d = nc.declare_dram_parameter("headb", [VC], F32, isOutput=False)
    if with_ln_affine:
        lng_d = nc.declare_dram_parameter("lng", [L, H], F32, isOutput=False)
        lnb_d = nc.declare_dram_parameter("lnb", [L, H], F32, isOutput=False)

    n_grp = n_steps // 4
    head_chunks = []
    v0 = 0
    while v0 < VC:
        head_chunks.append((v0, min(512, VC - v0)))
        v0 += 512

    with tile.TileContext(nc) as tc:
        with (
            tc.tile_pool(name="const", bufs=1) as const,
            tc.tile_pool(name="xmem", bufs=1) as xmem,
            tc.tile_pool(name="prep_ps", bufs=2, space="PSUM") as prep_ps,
            tc.tile_pool(name="ps_u", bufs=2, space="PSUM") as ps_u,
            tc.tile_pool(name="ps_cand", bufs=2, space="PSUM") as ps_cand,
            tc.tile_pool(name="ps_head", bufs=2, space="PSUM") as ps_head,
            tc.tile_pool(name="sb_state", bufs=4) as sb_state,
            tc.tile_pool(name="sb_tmp", bufs=3) as sb_tmp,
            tc.tile_pool(name="sb_small", bufs=3) as sb_small,
            tc.tile_pool(name="sb_stage", bufs=3) as sb_stage,
            tc.tile_pool(name="sb_out", bufs=2) as sb_out,
        ):
            # ---------------- prep: weights ----------------
            w1 = [[None] * 3 for _ in range(L)]
            for l in range(L):
                for c in range(3):
                    wf = const.tile([H, H + 1], F32, tag=f"w1f_{l}_{c}")
                    nc.sync.dma_start(wf[:], w1c_d[l, c])
                    wb = const.tile([H, H + 1], BF16, tag=f"w1b_{l}_{c}")
                    nc.vector.tensor_copy(wb[:], wf[:])
                    w1[l][c] = wb
            w2 = []
            for l in range(L):
                wf = const.tile([H, H], F32, tag=f"w2f_{l}")
                nc.sync.dma_start(wf[:], w2_d[l])
                wb = const.tile([H, H], BF16, tag=f"w2b_{l}")
                nc.vector.tensor_copy(wb[:], wf[:])
                w2.append(wb)
            hwf = const.tile([H, VC], F32, tag="hwf")
            nc.sync.dma_start(hwf[:], hw_d[:])
            hwb = const.tile([H, VC], BF16, tag="hwb")
            nc.vector.tensor_copy(hwb[:], hwf[:])

            if with_bias:
                b1f = const.tile([L, H + 1], F32, tag="b1f")
                nc.sync.dma_start(b1f[:], b1c_d[:])
                b1r = const.tile([L, H + 1], BF16, tag="b1r")
                nc.vector.tensor_copy(b1r[:], b1f[:])
                b2f = const.tile([L, H], F32, tag="b2f")
                nc.sync.dma_start(b2f[:], b2_d[:])
                b2r = const.tile([L, H], BF16, tag="b2r")
                nc.vector.tensor_copy(b2r[:], b2f[:])
                hbf = const.tile([1, VC], F32, tag="hbf")
                nc.sync.dma_start(hbf[:], hb_d[None, :])
                hbr = const.tile([1, VC], BF16, tag="hbr")
                nc.vector.tensor_copy(hbr[:], hbf[:])
                ones_col = const.tile([1, B], BF16, tag="ones_col")
                nc.vector.memset(ones_col[:], 1.0)
                ones_col128 = const.tile([1, 128], BF16, tag="ones_col128")
                nc.vector.memset(ones_col128[:], 1.0)
            if with_ln_affine:
                lng_bc, lnb_bc = [], []
                for l in range(L):
                    gb = const.tile([B, H], F32, tag=f"lng_{l}")
                    bb = const.tile([B, H], F32, tag=f"lnb_{l}")
                    g1 = const.tile([1, H], F32, tag=f"lng1_{l}")
                    b1t = const.tile([1, H], F32, tag=f"lnb1_{l}")
                    nc.sync.dma_start(g1[:], lng_d[l][None, :])
                    nc.sync.dma_start(b1t[:], lnb_d[l][None, :])
                    nc.gpsimd.partition_broadcast(gb[:], g1[:])
                    nc.gpsimd.partition_broadcast(bb[:], b1t[:])
                    lng_bc.append(gb)
                    lnb_bc.append(bb)

            ident = const.tile([128, 128], BF16, tag="ident")
            make_identity(nc, ident[:])
            magic = const.tile([B, 1], I32, tag="magic")
            nc.vector.memset(magic[:], MAGIC)
            c15 = const.tile([B, 1], F32, tag="c15")
            nc.vector.memset(c15[:], 1.5)
            cneghalf = const.tile([B, 1], F32, tag="cneghalf")
            nc.vector.memset(cneghalf[:], -0.5)

            # ---------------- prep: token stream ----------------
            # batch-major: xf[b, t, e]  (residual reads, partitions 0:32)
            xf = xmem.tile([B, n_steps, E], F32, tag="xf")
            nc.sync.dma_start(xf[:], xsb_d.rearrange("(t b) e -> b t e", b=B))
            # SB-major bf16 tiles for the transpose path
            xf2 = xmem.tile([128, n_grp, E], F32, tag="xf2")
            nc.sync.dma_start(xf2[:], xsb_d.rearrange("(g p) e -> p g e", p=128))
            xb = xmem.tile([128, n_grp, E], BF16, tag="xb")
            nc.vector.tensor_copy(xb[:], xf2[:])
            # transposed stream: xT[e, t*32 + b]  (partitions 0:64 always)
            xT = xmem.tile([E, n_steps * B], BF16, tag="xT")
            for g in range(n_grp):
                tps = prep_ps.tile([E, 128], BF16, tag="prep_t")
                nc.tensor.transpose(tps[:], xb[:, g, :], ident[:])
                nc.scalar.copy(xT[:, g * 128:(g + 1) * 128], tps[:])

            def x_lhsT(t):
                return xT[:, t * B:(t + 1) * B]

            # scale xf by 0.1 in place: its only consumer is the
            # layer-1 residual path (matmul x-chunks use xT instead)
            nc.scalar.mul(xf[:], xf[:], 0.1)

            # ---------------- state ----------------
            zero_hm = const.tile([B, H], BF16, tag="zero_hm")
            nc.vector.memset(zero_hm[:], 0.0)
            z1 = sb_state.tile([H, B], BF16, tag="hT_0")
            nc.vector.memset(z1[:], 0.0)
            z2 = sb_state.tile([H, B], BF16, tag="hT_0")
            nc.vector.memset(z2[:], 0.0)
            z3 = sb_stage.tile([H, 128], BF16, tag="h2T")
            nc.vector.memset(z3[:], 0.0)
            hT_prev = [z1[:], z3[:, 96:128]]
            hT_prev2 = [z2[:], z3[:, 64:96]]
            hm_prev = [zero_hm[:], zero_hm[:]]

            h2Tstage = None

            def newton_rsqrt(ssq, on_act=False):
                """rs = rsqrt(v); v [B,1] f32 > 0 (variance, eps skipped: var >= 9e-3).
                on_act: run the multiply chain on the Scalar engine (Copy-scale
                with per-partition APs) to offload DVE."""
                si = sb_small.tile([B, 1], I32, tag="nt_si")
                nc.vector.tensor_scalar(
                    si[:], ssq.bitcast(I32), 1, None,
                    op0=AL.logical_shift_right)
                yi = sb_small.tile([B, 1], I32, tag="nt_yi")
                nc.vector.tensor_tensor(yi[:], magic[:], si[:], op=AL.subtract)
                y = yi[:].bitcast(F32)
                if on_act:
                    # iteration on GpSimd (tensor_tensor only), off the
                    # DVE critical path; L2's chain has slack vs L1's.
                    vneg = sb_small.tile([B, 1], F32, tag="nt_vneg")
                    nc.gpsimd.tensor_tensor(vneg[:], ssq, cneghalf[:],
                                            op=AL.mult)
                    for it in range(NEWTON_ITERS):
                        y2 = sb_small.tile([B, 1], F32, tag="nt_y2")
                        nc.gpsimd.tensor_tensor(y2[:], y, y, op=AL.mult)
                        tq = sb_small.tile([B, 1], F32, tag="nt_tq")
                        nc.gpsimd.tensor_tensor(tq[:], y2[:], vneg[:],
                                                op=AL.mult)
                        w = sb_small.tile([B, 1], F32, tag="nt_w")
                        nc.gpsimd.tensor_tensor(w[:], tq[:], c15[:],
                                                op=AL.add)
                        yn = sb_small.tile([B, 1], F32, tag="nt_yn")
                        nc.gpsimd.tensor_tensor(yn[:], y, w[:], op=AL.mult)
                        y = yn[:]
                    return y
                for it in range(NEWTON_ITERS):
                    y2 = sb_small.tile([B, 1], F32, tag="nt_y2")
                    nc.vector.tensor_tensor(y2[:], y, y, op=AL.mult)
                    tq = sb_small.tile([B, 1], F32, tag="nt_tq")
                    nc.vector.tensor_scalar(
                        tq[:], y2[:], ssq, -0.5, op0=AL.mult, op1=AL.mult)
                    w = sb_small.tile([B, 1], F32, tag="nt_w")
                    nc.vector.tensor_scalar(
                        w[:], tq[:], 1.5, None, op0=AL.add)
                    yn = sb_small.tile([B, 1], F32, tag="nt_yn")
                    nc.vector.tensor_tensor(yn[:], y, w[:], op=AL.mult)
                    y = yn[:]
                return y

            def cell(l, t, x_lhsT_b, x_res_f32):
                """One layer-step. x_lhsT_b: [64,B] bf16 AP (stationary x chunk);
                x_res_f32: [B,64] f32 AP (residual input). Returns (hm, hT) APs."""
                u_ps = ps_u.tile([B, H + 1], F32, tag="u")
                nc.tensor.matmul(u_ps[:], x_lhsT_b, w1[l][2][:],
                                 start=True, stop=False)
                nc.tensor.matmul(u_ps[:], hT_prev2[l], w1[l][1][:],
                                 start=False, stop=False)
                nc.tensor.matmul(u_ps[:], hT_prev[l], w1[l][0][:],
                                 start=False, stop=not with_bias)
                if with_bias:
                    nc.tensor.matmul(u_ps[:], ones_col[:], b1r[l:l + 1, :],
                                     start=False, stop=True)

                ru = sb_tmp.tile([B, H], BF16, tag="ru")
                nc.scalar.activation(ru[:], u_ps[:, 0:H], AF.Relu)
                gt = sb_small.tile([B, 1], F32, tag="gate")
                nc.scalar.activation(gt[:], u_ps[:, H:H + 1], AF.Sigmoid)

                ruT_ps = prep_ps.tile([H, B], BF16, tag="prep_t")
                nc.tensor.transpose(ruT_ps[:], ru[:], ident[0:32, 0:32])
                ruT = sb_tmp.tile([H, B], BF16, tag="ruT")
                nc.scalar.copy(ruT[:], ruT_ps[:])

                cand_ps = ps_cand.tile([B, H], F32, tag="cand")
                nc.tensor.matmul(cand_ps[:], ruT[:], w2[l][:],
                                 start=True, stop=not with_bias)
                if with_bias:
                    nc.tensor.matmul(cand_ps[:], ones_col[:], b2r[l:l + 1, :],
                                     start=False, stop=True)

                base = sb_tmp.tile([B, H], F32, tag="base")
                if l == 0:
                    nc.gpsimd.tensor_tensor(base[:], x_res_f32, hm_prev[l],
                                            op=AL.add)
                else:
                    nc.vector.scalar_tensor_tensor(
                        base[:], x_res_f32, 0.1, hm_prev[l],
                        op0=AL.mult, op1=AL.add)

                p_t = sb_tmp.tile([B, H], F32, tag="p")
                nc.vector.scalar_tensor_tensor(
                    p_t[:], cand_ps[:], gt[:], base[:],
                    op0=AL.mult, op1=AL.add)

                bst = sb_small.tile([B, 6], F32, tag="bst")
                nc.vector.bn_stats(bst[:], p_t[:])
                agg = sb_small.tile([B, 2], F32, tag="agg")
                nc.vector.bn_aggr(agg[:], bst[:])
                rs = newton_rsqrt(agg[:, 1:2], on_act=(l == 1))

                hm = sb_state.tile([B, H], BF16, tag=f"hm_{l}")
                nc.vector.tensor_scalar(hm[:], p_t[:], agg[:, 0:1], rs,
                                        op0=AL.subtract, op1=AL.mult)
                if with_ln_affine:
                    hma = sb_state.tile([B, H], F32, tag=f"hma_{l}")
                    nc.vector.tensor_tensor(hma[:], hm[:], lng_bc[l][:],
                                            op=AL.mult)
                    hmb = sb_state.tile([B, H], BF16, tag=f"hmb_{l}")
                    nc.vector.tensor_tensor(hmb[:], hma[:], lnb_bc[l][:],
                                            op=AL.add)
                    hm = hmb
                hb = hm

                if l == 1:
                    c0 = 32 * (t % 4)
                    hT = h2Tstage[:, c0:c0 + 32]
                else:
                    hT_t = sb_state.tile([H, B], BF16, tag="hT_0")
                    hT = hT_t[:]
                nc.vector.transpose(hT[0:32, :], hb[:, 0:32])
                nc.vector.transpose(hT[32:64, :], hb[:, 32:64])

                hT_prev2[l] = hT_prev[l]
                hT_prev[l] = hT
                hm_prev[l] = hm[:]
                return hm[:], hT

            # ---------------- main loop ----------------
            for t in range(n_steps):
                g = t // 4
                if t % 4 == 0:
                    h2Tstage = sb_stage.tile([H, 128], BF16, tag="h2T")

                hm1, hT1 = cell(0, t, x_lhsT(t), xf[:, t, :])
                cell(1, t, hT1, hm1)

                if t % 4 == 3:
                    o_sb = sb_out.tile([128, VC], BF16, tag="osb")
                    for ki, (v0, vn) in enumerate(head_chunks):
                        hd_ps = ps_head.tile([128, 512], F32, tag="hd")
                        nc.tensor.matmul(hd_ps[:, 0:vn], h2Tstage[:],
                                         hwb[:, v0:v0 + vn],
                                         start=True, stop=not with_bias)
                        if with_bias:
                            nc.tensor.matmul(hd_ps[:, 0:vn], ones_col128[:],
                                             hbr[:, v0:v0 + vn],
                                             start=False, stop=True)
                        h1n = vn // 2
                        if ki % 8 < 3:
                            nc.vector.tensor_copy(o_sb[:, v0:v0 + h1n],
                                                  hd_ps[:, 0:h1n])
                            nc.vector.tensor_copy(o_sb[:, v0 + h1n:v0 + vn],
                                                  hd_ps[:, h1n:vn])
                        else:
                            nc.scalar.copy(o_sb[:, v0:v0 + h1n],
                                           hd_ps[:, 0:h1n])
                            nc.scalar.copy(o_sb[:, v0 + h1n:v0 + vn],
                                           hd_ps[:, h1n:vn])
                    dma_eng = nc.sync if g % 2 == 0 else nc.scalar
                    dma_eng.dma_start(
                        out_d[:, 4 * g:4 * g + 4, :].transpose([1, 0, 2]),
                        o_sb[:])

    nc.compile()
    return nc


def _get_nc(n_steps, with_bias, with_ln_affine):
    key = (n_steps, with_bias, with_ln_affine)
    if key not in _BUILD_CACHE:
        _BUILD_CACHE[key] = _build(n_steps, with_bias, with_ln_affine)
    return _BUILD_CACHE[key]


def _prep_inputs(input_ids, emb, W1, b1, W2, b2, Wg, bg, ln_g, ln_b,
                 headW, headb, n_steps):
    input_ids = np.asarray(input_ids)
    emb = np.asarray(emb, np.float32)
    W1 = np.asarray(W1, np.float32)
    Wg = np.asarray(Wg, np.float32)
    W2 = np.asarray(W2, np.float32)
    b1 = np.asarray(b1, np.float32)
    bg = np.asarray(bg, np.float32)
    b2 = np.asarray(b2, np.float32)
    ln_g = np.asarray(ln_g, np.float32)
    ln_b = np.asarray(ln_b, np.float32)
    headW = np.asarray(headW, np.float32)
    headb = np.asarray(headb, np.float32)

    x = emb[input_ids]  # [B, S, E]
    xsb = np.ascontiguousarray(
        x[:, :n_steps, :].transpose(1, 0, 2).reshape(n_steps * B, E))

    w1cat = np.concatenate([W1, Wg], axis=2)  # [L, 192, 65]
    w1c = np.stack([w1cat[:, 0:64], w1cat[:, 64:128], w1cat[:, 128:192]],
                   axis=1)  # [L, 3, 64, 65]
    b1c = np.concatenate([b1, bg], axis=1)  # [L, 65]

    with_bias = bool(np.any(b1c) or np.any(b2) or np.any(headb))
    with_ln = bool(np.any(ln_g != 1.0) or np.any(ln_b))

    base = {
        "xsb": xsb,
        "w1c": np.ascontiguousarray(w1c),
        "w2": np.ascontiguousarray(W2),
    }
    if with_bias:
        base["b1c"] = np.ascontiguousarray(b1c)
        base["b2v"] = np.ascontiguousarray(b2)
    if with_ln:
        base["lng"] = np.ascontiguousarray(ln_g)
        base["lnb"] = np.ascontiguousarray(ln_b)

    in_maps = []
    for c in range(NCORES):
        m = dict(base)
        m["headw"] = np.ascontiguousarray(headW[:, c * VC:(c + 1) * VC])
        if with_bias:
            m["headb"] = np.ascontiguousarray(headb[c * VC:(c + 1) * VC])
        in_maps.append(m)
    return in_maps, with_bias, with_ln


def _run(inputs, n_steps=S, trace=False):
    in_maps, with_bias, with_ln = _prep_inputs(n_steps=n_steps, **inputs)
    nc = _get_nc(n_steps, with_bias, with_ln)
    res = run_bass_kernel_spmd(nc, in_maps, core_ids=list(range(NCORES)),
                               trace=trace)
    outs = [np.asarray(res.results[i]["out"]).astype(np.float32)
            for i in range(NCORES)]
    full = np.concatenate(outs, axis=2)  # [B, n_steps, V]
    return full, res


def kernel(**inputs):
    out, _ = _run(inputs, n_steps=S, trace=False)
    return out


def run_traced(**inputs):
    """For test.py: returns (out, BassKernelResults with exec_time_ns)."""
    return _run(inputs, n_steps=S, trace=True)


def run_steps(n_steps, **inputs):
    """Debug helper: run a truncated sequence."""
    out, res = _run(inputs, n_steps=n_steps, trace=False)
    return out, res


# revision 15
# speedup vs baseline: 1.0409x; 1.0409x over previous
"""Trainium2 Bass kernel for nn_ARSLMModel (2-layer gated recurrent LM).

Strategy (8 NeuronCores):
  - The head output [B,S,V] (1 GB fp32) dominates -> shard the vocab dim
    across cores (V/8 = 4000 per core). Host concatenates.
  - The 256-step recurrence is tiny compute but strictly sequential; it is
    replicated on every core (sharding batch would not reduce per-step
    instruction count) and overlapped with the head matmul + output DMA.
  - Matmuls run in bf16 (validated: end-to-end l2 rel err ~5e-3 vs fp32
    reference, gate 2e-2); all elementwise math in fp32.
  - LayerNorm rsqrt via bit-trick seed + Newton iterations on DVE (ACT table
    stays resident in the sigmoid set: relu/sigmoid/square/copy only).

Layouts:
  - Hidden state kept B-major [32, 64] for all elementwise/LN work; matmuls
    take the transposed state [64, 32] (DVE 32x32 stream transposes) as the
    stationary operand so outputs land B-major.
  - Layer-2 transposed states are written straight into a [64, 128] staging
    tile covering 4 timesteps; that tile IS the head matmul's stationary
    operand (rows r = (t%4)*32 + b match the SB-major output order).
"""

import numpy as np

import concourse.bass as bass
import concourse.mybir as mybir
from concourse import bacc, tile
from concourse.masks import make_identity
from concourse.bass_utils import run_bass_kernel_spmd

V, E, H, L = 32000, 64, 64, 2
B, S = 32, 256
NCORES = 8
VC = V // NCORES

F32 = mybir.dt.float32
BF16 = mybir.dt.bfloat16
I32 = mybir.dt.int32
AL = mybir.AluOpType
AF = mybir.ActivationFunctionType

NEWTON_ITERS = 1
MAGIC = 0x5F3759DF

_BUILD_CACHE = {}


def _build(n_steps, with_bias, with_ln_affine):
    """Build the SPMD single-core graph (all cores run the same program)."""
    nc = bacc.Bacc()

    xsb_d = nc.declare_dram_parameter("xsb", [n_steps * B, E], F32, isOutput=False)
    w1c_d = nc.declare_dram_parameter("w1c", [L, 3, H, H + 1], F32, isOutput=False)
    w2_d = nc.declare_dram_parameter("w2", [L, H, H], F32, isOutput=False)
    hw_d = nc.declare_dram_parameter("headw", [H, VC], F32, isOutput=False)
    out_d = nc.declare_dram_parameter("out", [B, n_steps, VC], BF16, isOutput=True)
    if with_bias:
        b1c_d = nc.declare_dram_parameter("b1c", [L, H + 1], F32, isOutput=False)
        b2_d = nc.declare_dram_parameter("b2v", [L, H], F32, isOutput=False)
        hb_d = nc.declare_dram_parameter("headb", [VC], F32, isOutput=False)
    if with_ln_affine:
        lng_d = nc.declare_dram_parameter("lng", [L, H], F32, isOutput=False)
        lnb_d = nc.declare_dram_parameter("lnb", [L, H], F32, isOutput=False)

    n_grp = n_steps // 4
    head_chunks = []
    v0 = 0
    while v0 < VC:
        head_chunks.append((v0, min(512, VC - v0)))
        v0 += 512

    with tile.TileContext(nc) as tc:
        with (
            tc.tile_pool(name="const", bufs=1) as const,
            tc.tile_pool(name="xmem", bufs=1) as xmem,
            tc.tile_pool(name="prep_ps", bufs=2, space="PSUM") as prep_ps,
            tc.tile_pool(name="ps_u", bufs=2, space="PSUM") as ps_u,
            tc.tile_pool(name="ps_cand", bufs=2, space="PSUM") as ps_cand,
            tc.tile_pool(name="ps_head", bufs=2, space="PSUM") as ps_head,
            tc.tile_pool(name="sb_state", bufs=4) as sb_state,
            tc.tile_pool(name="sb_tmp", bufs=3) as sb_tmp,
            tc.tile_pool(name="sb_small", bufs=3) as sb_small,
            tc.tile_pool(name="sb_stage", bufs=3) as sb_stage,
            tc.tile_pool(name="sb_out", bufs=2) as sb_out,
        ):
            # ---------------- prep: weights ----------------
            w1 = [[None] * 3 for _ in range(L)]
            for l in range(L):
                for c in range(3):
                    wf = const.tile([H, H + 1], F32, tag=f"w1f_{l}_{c}")
                    nc.sync.dma_start(wf[:], w1c_d[l, c])
                    wb = const.tile([H, H + 1], BF16, tag=f"w1b_{l}_{c}")
                    nc.vector.tensor_copy(wb[:], wf[:])
                    w1[l][c] = wb
            w2 = []
            for l in range(L):
                wf = const.tile([H, H], F32, tag=f"w2f_{l}")
                nc.sync.dma_start(wf[:], w2_d[l])
                wb = const.tile([H, H], BF16, tag=f"w2b_{l}")
                nc.vector.tensor_copy(wb[:], wf[:])
                w2.append(wb)
            hwf = const.tile([H, VC], F32, tag="hwf")
            nc.sync.dma_start(hwf[:], hw_d[:])
            hwb = const.tile([H, VC], BF16, tag="hwb")
            nc.vector.tensor_copy(hwb[:], hwf[:])

            if with_bias:
                b1f = const.tile([L, H + 1], F32, tag="b1f")
                nc.sync.dma_start(b1f[:], b1c_d[:])
                b1r = const.tile([L, H + 1], BF16, tag="b1r")
                nc.vector.tensor_copy(b1r[:], b1f[:])
                b2f = const.tile([L, H], F32, tag="b2f")
                nc.sync.dma_start(b2f[:], b2_d[:])
                b2r = const.tile([L, H], BF16, tag="b2r")
                nc.vector.tensor_copy(b2r[:], b2f[:])
                hbf = const.tile([1, VC], F32, tag="hbf")
                nc.sync.dma_start(hbf[:], hb_d[None, :])
                hbr = const.tile([1, VC], BF16, tag="hbr")
                nc.vector.tensor_copy(hbr[:], hbf[:])
                ones_col = const.tile([1, B], BF16, tag="ones_col")
                nc.vector.memset(ones_col[:], 1.0)
                ones_col128 = const.tile([1, 128], BF16, tag="ones_col128")
                nc.vector.memset(ones_col128[:], 1.0)
            if with_ln_affine:
                lng_bc, lnb_bc = [], []
                for l in range(L):
                    gb = const.tile([B, H], F32, tag=f"lng_{l}")
                    bb = const.tile([B, H], F32, tag=f"lnb_{l}")
                    g1 = const.tile([1, H], F32, tag=f"lng1_{l}")
                    b1t = const.tile([1, H], F32, tag=f"lnb1_{l}")
                    nc.sync.dma_start(g1[:], lng_d[l][None, :])
                    nc.sync.dma_start(b1t[:], lnb_d[l][None, :])
                    nc.gpsimd.partition_broadcast(gb[:], g1[:])
                    nc.gpsimd.partition_broadcast(bb[:], b1t[:])
                    lng_bc.append(gb)
                    lnb_bc.append(bb)

            ident = const.tile([128, 128], BF16, tag="ident")
            make_identity(nc, ident[:])
            magic = const.tile([B, 1], I32, tag="magic")
            nc.vector.memset(magic[:], MAGIC)
            c15 = const.tile([B, 1], F32, tag="c15")
            nc.vector.memset(c15[:], 1.5)
            cneghalf = const.tile([B, 1], F32, tag="cneghalf")
            nc.vector.memset(cneghalf[:], -0.5)

            # ---------------- prep: token stream ----------------
            # batch-major: xf[b, t, e]  (residual reads, partitions 0:32)
            xf = xmem.tile([B, n_steps, E], F32, tag="xf")
            nc.sync.dma_start(xf[:], xsb_d.rearrange("(t b) e -> b t e", b=B))
            # SB-major bf16 tiles for the transpose path
            xf2 = xmem.tile([128, n_grp, E], F32, tag="xf2")
            nc.sync.dma_start(xf2[:], xsb_d.rearrange("(g p) e -> p g e", p=128))
            xb = xmem.tile([128, n_grp, E], BF16, tag="xb")
            nc.vector.tensor_copy(xb[:], xf2[:])
            # transposed stream: xT[e, t*32 + b]  (partitions 0:64 always)
            xT = xmem.tile([E, n_steps * B], BF16, tag="xT")
            for g in range(n_grp):
                tps = prep_ps.tile([E, 128], BF16, tag="prep_t")
                nc.tensor.transpose(tps[:], xb[:, g, :], ident[:])
                nc.scalar.copy(xT[:, g * 128:(g + 1) * 128], tps[:])

            def x_lhsT(t):
                return xT[:, t * B:(t + 1) * B]

            # scale xf by 0.1 in place: its only consumer is the
            # layer-1 residual path (matmul x-chunks use xT instead)
            nc.scalar.mul(xf[:], xf[:], 0.1)

            # ---------------- state ----------------
            zero_hm = const.tile([B, H], BF16, tag="zero_hm")
            nc.vector.memset(zero_hm[:], 0.0)
            z1 = sb_state.tile([H, B], BF16, tag="hT_0")
            nc.vector.memset(z1[:], 0.0)
            z2 = sb_state.tile([H, B], BF16, tag="hT_0")
            nc.vector.memset(z2[:], 0.0)
            z3 = sb_stage.tile([H, 128], BF16, tag="h2T")
            nc.vector.memset(z3[:], 0.0)
            hT_prev = [z1[:], z3[:, 96:128]]
            hT_prev2 = [z2[:], z3[:, 64:96]]
            hm_prev = [zero_hm[:], zero_hm[:]]

            h2Tstage = None

            def newton_rsqrt(ssq, on_act=False):
                """rs = rsqrt(v); v [B,1] f32 > 0 (variance, eps skipped: var >= 9e-3).
                on_act: run the multiply chain on the Scalar engine (Copy-scale
                with per-partition APs) to offload DVE."""
                si = sb_small.tile([B, 1], I32, tag="nt_si")
                nc.vector.tensor_scalar(
                    si[:], ssq.bitcast(I32), 1, None,
                    op0=AL.logical_shift_right)
                yi = sb_small.tile([B, 1], I32, tag="nt_yi")
                nc.vector.tensor_tensor(yi[:], magic[:], si[:], op=AL.subtract)
                y = yi[:].bitcast(F32)
                if on_act:
                    # iteration on GpSimd (tensor_tensor only), off the
                    # DVE critical path; L2's chain has slack vs L1's.
                    vneg = sb_small.tile([B, 1], F32, tag="nt_vneg")
                    nc.gpsimd.tensor_tensor(vneg[:], ssq, cneghalf[:],
                                            op=AL.mult)
                    for it in range(NEWTON_ITERS):
                        y2 = sb_small.tile([B, 1], F32, tag="nt_y2")
                        nc.gpsimd.tensor_tensor(y2[:], y, y, op=AL.mult)
                        tq = sb_small.tile([B, 1], F32, tag="nt_tq")
                        nc.gpsimd.tensor_tensor(tq[:], y2[:], vneg[:],
                                                op=AL.mult)
                        w = sb_small.tile([B, 1], F32, tag="nt_w")
                        nc.gpsimd.tensor_tensor(w[:], tq[:], c15[:],
                                                op=AL.add)
                        yn = sb_small.tile([B, 1], F32, tag="nt_yn")
                        nc.gpsimd.tensor_tensor(yn[:], y, w[:], op=AL.mult)
                        y = yn[:]
                    return y
                for it in range(NEWTON_ITERS):
                    y2 = sb_small.tile([B, 1], F32, tag="nt_y2")
                    nc.vector.tensor_tensor(y2[:], y, y, op=AL.mult)
                    tq = sb_small.tile([B, 1], F32, tag="nt_tq")
                    nc.vector.tensor_scalar(
                        tq[:], y2[:], ssq, -0.5, op0=AL.mult, op1=AL.mult)
                    w = sb_small.tile([B, 1], F32, tag="nt_w")
                    nc.vector.tensor_scalar(
                        w[:], tq[:], 1.5, None, op0=AL.add)
                    yn = sb_small.tile([B, 1], F32, tag="nt_yn")
                    nc.vector.tensor_tensor(yn[:], y, w[:], op=AL.mult)
                    y = yn[:]
                return y

            def cell(l, t, x_lhsT_b, x_res_f32):
                """One layer-step. x_lhsT_b: [64,B] bf16 AP (stationary x chunk);
                x_res_f32: [B,64] f32 AP (residual input). Returns (hm, hT) APs."""
                u_ps = ps_u.tile([B, H + 1], F32, tag="u")
                nc.tensor.matmul(u_ps[:], x_lhsT_b, w1[l][2][:],
                                 start=True, stop=False)
                nc.tensor.matmul(u_ps[:], hT_prev2[l], w1[l][1][:],
                                 start=False, stop=False)
                nc.tensor.matmul(u_ps[:], hT_prev[l], w1[l][0][:],
                                 start=False, stop=not with_bias)
                if with_bias:
                    nc.tensor.matmul(u_ps[:], ones_col[:], b1r[l:l + 1, :],
                                     start=False, stop=True)

                ru = sb_tmp.tile([B, H], BF16, tag="ru")
                nc.scalar.activation(ru[:], u_ps[:, 0:H], AF.Relu)
                gt = sb_small.tile([B, 1], F32, tag="gate")
                nc.scalar.activation(gt[:], u_ps[:, H:H + 1], AF.Sigmoid)

                ruT_ps = prep_ps.tile([H, B], BF16, tag="prep_t")
                nc.tensor.transpose(ruT_ps[:], ru[:], ident[0:32, 0:32])
                ruT = sb_tmp.tile([H, B], BF16, tag="ruT")
                nc.scalar.copy(ruT[:], ruT_ps[:])

                cand_ps = ps_cand.tile([B, H], F32, tag="cand")
                nc.tensor.matmul(cand_ps[:], ruT[:], w2[l][:],
                                 start=True, stop=not with_bias)
                if with_bias:
                    nc.tensor.matmul(cand_ps[:], ones_col[:], b2r[l:l + 1, :],
                                     start=False, stop=True)

                base = sb_tmp.tile([B, H], F32, tag="base")
                if l == 0:
                    nc.gpsimd.tensor_tensor(base[:], x_res_f32, hm_prev[l],
                                            op=AL.add)
                else:
                    nc.vector.scalar_tensor_tensor(
                        base[:], x_res_f32, 0.1, hm_prev[l],
                        op0=AL.mult, op1=AL.add)

                p_t = sb_tmp.tile([B, H], F32, tag="p")
                nc.vector.scalar_tensor_tensor(
                    p_t[:], cand_ps[:], gt[:], base[:],
                    op0=AL.mult, op1=AL.add)

                bst = sb_small.tile([B, 6], F32, tag="bst")
                nc.vector.bn_stats(bst[:], p_t[:])
                agg = sb_small.tile([B, 2], F32, tag="agg")
                nc.vector.bn_aggr(agg[:], bst[:])
                rs = newton_rsqrt(agg[:, 1:2])

                hm = sb_state.tile([B, H], BF16, tag=f"hm_{l}")
                nc.vector.tensor_scalar(hm[:], p_t[:], agg[:, 0:1], rs,
                                        op0=AL.subtract, op1=AL.mult)
                if with_ln_affine:
                    hma = sb_state.tile([B, H], F32, tag=f"hma_{l}")
                    nc.vector.tensor_tensor(hma[:], hm[:], lng_bc[l][:],
                                            op=AL.mult)
                    hmb = sb_state.tile([B, H], BF16, tag=f"hmb_{l}")
                    nc.vector.tensor_tensor(hmb[:], hma[:], lnb_bc[l][:],
                                            op=AL.add)
                    hm = hmb
                hb = hm

                if l == 1:
                    c0 = 32 * (t % 4)
                    hT = h2Tstage[:, c0:c0 + 32]
                else:
                    hT_t = sb_state.tile([H, B], BF16, tag="hT_0")
                    hT = hT_t[:]
                nc.vector.transpose(hT[0:32, :], hb[:, 0:32])
                nc.vector.transpose(hT[32:64, :], hb[:, 32:64])

                hT_prev2[l] = hT_prev[l]
                hT_prev[l] = hT
                hm_prev[l] = hm[:]
                return hm[:], hT

            # ---------------- main loop ----------------
            for t in range(n_steps):
                g = t // 4
                if t % 4 == 0:
                    h2Tstage = sb_stage.tile([H, 128], BF16, tag="h2T")

                hm1, hT1 = cell(0, t, x_lhsT(t), xf[:, t, :])
                cell(1, t, hT1, hm1)

                if t % 4 == 3:
                    o_sb = sb_out.tile([128, VC], BF16, tag="osb")
                    for ki, (v0, vn) in enumerate(head_chunks):
                        hd_ps = ps_head.tile([128, 512], F32, tag="hd")
                        nc.tensor.matmul(hd_ps[:, 0:vn], h2Tstage[:],
                                         hwb[:, v0:v0 + vn],
                                         start=True, stop=not with_bias)
                        if with_bias:
                            nc.tensor.matmul(hd_ps[:, 0:vn], ones_col128[:],
                                             hbr[:, v0:v0 + vn],
                                             start=False, stop=True)
                        h1n = vn // 2
                        if ki % 8 < 3:
                            nc.vector.tensor_copy(o_sb[:, v0:v0 + h1n],
                                                  hd_ps[:, 0:h1n])
                            nc.vector.tensor_copy(o_sb[:, v0 + h1n:v0 + vn],
                                                  hd_ps[:, h1n:vn])
                        else:
                            nc.scalar.copy(o_sb[:, v0:v0 + h1n],
                                           hd_ps[:, 0:h1n])
                            nc.scalar.copy(o_sb[:, v0 + h1n:v0 + vn],
                                           hd_ps[:, h1n:vn])
                    dma_eng = nc.sync if g % 2 == 0 else nc.scalar
                    dma_eng.dma_start(
                        out_d[:, 4 * g:4 * g + 4, :].transpose([1, 0, 2]),
                        o_sb[:])

    nc.compile()
    return nc


def _get_nc(n_steps, with_bias, with_ln_affine):
    key = (n_steps, with_bias, with_ln_affine)
    if key not in _BUILD_CACHE:
        _BUILD_CACHE[key] = _build(n_steps, with_bias, with_ln_affine)
    return _BUILD_CACHE[key]


def _prep_inputs(input_ids, emb, W1, b1, W2, b2, Wg, bg, ln_g, ln_b,
                 headW, headb, n_steps):
    input_ids = np.asarray(input_ids)
    emb = np.asarray(emb, np.float32)
    W1 = np.asarray(W1, np.float32)
    Wg = np.asarray(Wg, np.float32)
    W2 = np.asarray(W2, np.float32)
    b1 = np.asarray(b1, np.float32)
    bg = np.asarray(bg, np.float32)
    b2 = np.asarray(b2, np.float32)
    ln_g = np.asarray(ln_g, np.float32)
    ln_b = np.asarray(ln_b, np.float32)
    headW = np.asarray(headW, np.float32)
    headb = np.asarray(headb, np.float32)

    x = emb[input_ids]  # [B, S, E]
    xsb = np.ascontiguousarray(
        x[:, :n_steps, :].transpose(1, 0, 2).reshape(n_steps * B, E))

    w1cat = np.concatenate([W1, Wg], axis=2)  # [L, 192, 65]
    w1c = np.stack([w1cat[:, 0:64], w1cat[:, 64:128], w1cat[:, 128:192]],
                   axis=1)  # [L, 3, 64, 65]
    b1c = np.concatenate([b1, bg], axis=1)  # [L, 65]

    with_bias = bool(np.any(b1c) or np.any(b2) or np.any(headb))
    with_ln = bool(np.any(ln_g != 1.0) or np.any(ln_b))

    base = {
        "xsb": xsb,
        "w1c": np.ascontiguousarray(w1c),
        "w2": np.ascontiguousarray(W2),
    }
    if with_bias:
        base["b1c"] = np.ascontiguousarray(b1c)
        base["b2v"] = np.ascontiguousarray(b2)
    if with_ln:
        base["lng"] = np.ascontiguousarray(ln_g)
        base["lnb"] = np.ascontiguousarray(ln_b)

    in_maps = []
    for c in range(NCORES):
        m = dict(base)
        m["headw"] = np.ascontiguousarray(headW[:, c * VC:(c + 1) * VC])
        if with_bias:
            m["headb"] = np.ascontiguousarray(headb[c * VC:(c + 1) * VC])
        in_maps.append(m)
    return in_maps, with_bias, with_ln


def _run(inputs, n_steps=S, trace=False):
    in_maps, with_bias, with_ln = _prep_inputs(n_steps=n_steps, **inputs)
    nc = _get_nc(n_steps, with_bias, with_ln)
    res = run_bass_kernel_spmd(nc, in_maps, core_ids=list(range(NCORES)),
                               trace=trace)
    outs = [np.asarray(res.results[i]["out"]).astype(np.float32)
            for i in range(NCORES)]
    full = np.concatenate(outs, axis=2)  # [B, n_steps, V]
    return full, res


def kernel(**inputs):
    out, _ = _run(inputs, n_steps=S, trace=False)
    return out


def run_traced(**inputs):
    """For test.py: returns (out, BassKernelResults with exec_time_ns)."""
    return _run(inputs, n_steps=S, trace=True)


def run_steps(n_steps, **inputs):
    """Debug helper: run a truncated sequence."""
    out, res = _run(inputs, n_steps=n_steps, trace=False)
    return out, res
